# revision 34
# baseline (speedup 1.0000x reference)
"""Trainium2 Bass kernel: GNN message passing (metasurface inverse design).

Distribution (8 NeuronCores):
  - Edges sharded by target-node range (12500 nodes per core).
  - Node state h replicated each layer via AllGather of the per-core shard.
  - Per-edge gather h[src] uses the SWDGE dma_gather extended instruction
    (512B rows); edges are gathered in (src-chunk, tgt-block, tgt) order.
  - The per-target scatter-add is done on the TensorEngine: for each
    128-edge chunk a one-hot matrix S[j, t] (DVE is_equal against an iota
    row) and matmul(psum[f, t] += E_chunk^T. S) accumulate the segment sum
    per 128-target block.  No SWDGE scatter, no duplicate-target rounds,
    no agg zeroing; the aggregation lands feature-major in PSUM, which is
    exactly the layout the dense phase consumes (saves the agg transposes).

Math restructuring (linearity of the edge matmul over the segment-sum):
    reference: agg = segsum_tgt(concat(h[src], h[tgt]) @ Wm[l] + bm[l])
    here:      G   = segsum_tgt(h[src])            (gather + PE one-hot matmul)
               agg = G @ W1 + deg * (h @ W2) + deg x bm
  with Wm[l] = [W1; W2] and deg = in-degree.  All FLOPs are dense matmuls.
"""

import os

import ml_dtypes
import numpy as np

import concourse.bacc as bacc
import concourse.bass as bass
import concourse.mybir as mybir
import concourse.tile as tile
from concourse.bass_utils import run_bass_kernel_spmd
from concourse.masks import make_identity

F32 = mybir.dt.float32
BF16 = mybir.dt.bfloat16
I16 = mybir.dt.int16
NCORES = 8
H = 128
P = 128
SG = 16  # chunks per one-hot generation slab
TWO_PI = 6.283185307179586


class Cfg:
    def __init__(self, N, E, B, M, L, chunk=25000, blk=8192, wblk=4):
        assert N % NCORES == 0
        self.N, self.E, self.B, self.M, self.L = N, E, B, M, L
        self.chunk = chunk
        self.wblk = wblk  # tgt blocks (128 nodes) per window == dense tile
        self.ns = N // NCORES
        self.ns_pad = ((self.ns + 511) // 512) * 512
        self.nk = N // chunk  # src chunks (int16 gather index range)
        assert chunk <= 32768


def _pack_v2(cfg, src, tgt):
    """Slot layout: per core, stream-major (src chunk k), then tgt block b,
    then tgt.  Each (k, b) group padded to a multiple of 128 slots; group
    sizes are max over cores (SPMD: identical program on all cores).

    Returns gidx [NCORES, 128, TOT//16] int16 (gather indices, 16-wrapped),
    tcol [NCORES, 128, TOT//128] f32 (per chunk, per lane: tgt col within
    block, or -1 for padding), windows (shared call/chunk structure), and
    rowcap (max 128-slot rows per gather call).
    """
    NS, CH, NK, WB = cfg.ns, cfg.chunk, cfg.nk, cfg.wblk
    NBLK = cfg.ns_pad // 128
    HS = NS // 2  # AllGather half size (6250)
    core = tgt // NS
    lt = (tgt - core * NS).astype(np.int64)
    # source rows permuted into (half, core, r%half) order so the AllGather
    # can be split into two overlapped collectives (h_fullA / h_fullB)
    sc = src // NS
    sr = src - sc * NS
    prow = np.where(sr < HS, sc * HS + sr, sc * HS + (sr - HS) + NCORES * HS)
    k = prow // CH
    ls = (prow - k * CH).astype(np.int16)
    b = lt // 128

    key = (core * NK + k) * NBLK + b
    cnt = np.bincount(key, minlength=NCORES * NK * NBLK).reshape(NCORES, NK, NBLK)
    nmax = cnt.max(axis=0)  # [NK, NBLK]
    Pkb = ((nmax + 127) // 128) * 128
    Pkb[0, Pkb.sum(axis=0) == 0] = 128  # every block gets >= 1 chunk

    S_k = Pkb.sum(axis=1)
    base_k = np.concatenate([[0], np.cumsum(S_k)[:-1]])
    off = base_k[:, None] + np.concatenate(
        [np.zeros((NK, 1), np.int64), np.cumsum(Pkb, axis=1)[:, :-1]], axis=1
    )
    TOT = int(S_k.sum())

    o = np.lexsort((lt, key))
    st = np.concatenate([[0], np.cumsum(cnt.reshape(-1))[:-1]])
    rank = np.empty(len(tgt), np.int64)
    rank[o] = np.arange(len(tgt)) - st[key[o]]
    slot = off[k, b] + rank

    gidx = np.zeros((NCORES, 16, TOT // 16), np.int16)
    gidx[core, slot % 16, slot // 16] = ls
    gidx = np.tile(gidx, (1, 8, 1))
    tcol = np.full((NCORES, 128, TOT // 128), -1.0, np.float32)
    tcol[core, slot % 128, slot // 128] = (lt - b * 128).astype(np.float32)

    NW = NBLK // WB
    windows = []
    perm = []  # window-chunk order -> global chunk index (tcol column)
    for w in range(NW):
        blocks = list(range(w * WB, (w + 1) * WB))
        per_k = []
        for kk in range(NK):
            s0 = int(off[kk, blocks[0]])
            nsl = int(Pkb[kk, blocks].sum())
            per_k.append((s0, nsl))
        blist = []
        for bb in blocks:
            ch = []
            for kk in range(NK):
                s0k = per_k[kk][0]
                g0 = int(off[kk, bb])
                for i in range(int(Pkb[kk, bb]) // 128):
                    ch.append((kk, (g0 - s0k) // 128 + i))
                    perm.append(g0 // 128 + i)
            blist.append(ch)
        windows.append((per_k, blist))
    tcol = tcol[:, :, np.array(perm)]  # reindex to window-chunk order
    rowcap = max(nsl for per_k, _ in windows for _, nsl in per_k) // 128
    return gidx, tcol, windows, rowcap, TOT


def _build(cfg, TOT, windows, rowcap):
    """Emit the Bass/Tile program (identical for all cores)."""
    phase = os.environ.get("GNN_PHASE", "full")
    NS, NSP, B, M, L = cfg.ns, cfg.ns_pad, cfg.B, cfg.M, cfg.L
    NT = NSP // 512  # dense tiles (512 nodes each)
    NBLK = NSP // 128
    MT = M // 128
    NK = cfg.nk
    assert len(windows) == NT and cfg.wblk == 4

    nqueues = int(os.environ.get("GNN_QUEUES", "4"))
    nc = bacc.Bacc(None, num_devices=NCORES, num_swdge_queues=nqueues)

    x_s = nc.dram_tensor("x_s", [NSP, H], F32, kind="ExternalInput")
    Wemb = nc.dram_tensor("Wemb", [H, H], F32, kind="ExternalInput")
    bemb = nc.dram_tensor("bemb", [H, 1], F32, kind="ExternalInput")
    Wm = nc.dram_tensor("Wm", [L, 2 * H, H], F32, kind="ExternalInput")
    bmv = nc.dram_tensor("bmv", [L, 1, H], F32, kind="ExternalInput")
    W1r = nc.dram_tensor("W1r", [H, 2 * H], F32, kind="ExternalInput")
    b1r = nc.dram_tensor("b1r", [H, 2], F32, kind="ExternalInput")
    W2r = nc.dram_tensor("W2r", [2 * H, M], F32, kind="ExternalInput")
    b2r = nc.dram_tensor("b2r", [H, MT], F32, kind="ExternalInput")
    gidx = nc.dram_tensor("gidx", [P, TOT // 16], I16, kind="ExternalInput")
    Sslab = nc.dram_tensor(
        "Sslab", [P, (TOT // 128) * P], BF16, kind="ExternalInput"
    )
    deg_pb = nc.dram_tensor("deg_pb", [P, NBLK], F32, kind="ExternalInput")
    degT = nc.dram_tensor("degT", [1, NSP], F32, kind="ExternalInput")
    ind = nc.dram_tensor("ind", [P, NBLK * B], F32, kind="ExternalInput")

    outT = nc.dram_tensor("outT", [M, B], F32, kind="ExternalOutput")

    HS = NS // 2  # AllGather half (6250 rows per core)
    # ping-pong per layer so a mid-loop AllGather never overwrites the
    # buffer still being gathered from
    h_fullA = [
        nc.dram_tensor(f"h_fullA{i}", [NCORES * HS, H], BF16, addr_space="Shared")
        for i in range(2)
    ]
    h_fullB = [
        nc.dram_tensor(f"h_fullB{i}", [NCORES * HS, H], BF16, addr_space="Shared")
        for i in range(2)
    ]
    h_sh = nc.dram_tensor("h_sh", [NSP, H], BF16)
    h_shF = nc.dram_tensor("h_shF", [NSP, H], F32)
    gpart = nc.dram_tensor("gpart", [H, B], F32)
    gsum = nc.dram_tensor("gsum", [H, B], F32, addr_space="Shared")

    groups = [list(range(NCORES))]

    with tile.TileContext(nc) as tc:
        with (
            tc.tile_pool(name="const", bufs=1) as cp,
            tc.tile_pool(name="work", bufs=2) as wp,
            tc.tile_pool(name="sgen", bufs=4) as sp,
            tc.tile_pool(name="slab", bufs=3) as slb,
        ):
            pp_cm = tc.tile_pool(name="psum", bufs=2, space="PSUM")
            pp = pp_cm.__enter__()
            ep_cm = tc.tile_pool(name="edge", bufs=3)
            ep = ep_cm.__enter__()
            # ---- persistent constants in SBUF ----
            ident = cp.tile([P, P], F32, tag="ident")
            make_identity(nc, ident[:])
            Wemb_sb = cp.tile([H, H], F32, tag="wemb")
            nc.sync.dma_start(Wemb_sb[:], Wemb[:])
            bemb_sb = cp.tile([H, 1], F32, tag="bemb")
            nc.sync.dma_start(bemb_sb[:], bemb[:])
            W1_sb, W2_sb, bm_sb = [], [], []
            for l in range(L):
                w1 = cp.tile([H, H], F32, tag=f"w1_{l}")
                nc.sync.dma_start(w1[:], Wm[l, :H, :])
                w2 = cp.tile([H, H], F32, tag=f"w2_{l}")
                nc.sync.dma_start(w2[:], Wm[l, H:, :])
                bm_t = cp.tile([1, H], F32, tag=f"bm_{l}")
                nc.sync.dma_start(bm_t[:], bmv[l, :, :])
                W1_sb.append(w1)
                W2_sb.append(w2)
                bm_sb.append(bm_t)
            deg_sb = cp.tile([P, NBLK], F32, tag="deg")
            nc.sync.dma_start(deg_sb[:], deg_pb[:])


            def load_nm(pool, dram_rows, tag, dt=F32):
                """Load 512 node-major rows as [128, 4, 128] (block, feat)."""
                t = pool.tile([P, 4 * H], dt, tag=tag)
                nc.sync.dma_start(
                    t[:].rearrange("p (b f) -> p b f", f=H),
                    dram_rows.rearrange("(b p) f -> p b f", p=P),
                )
                return t

            def store_nm(t, dram_rows):
                nc.sync.dma_start(
                    dram_rows.rearrange("(b p) f -> p b f", p=P),
                    t[:].rearrange("p (b f) -> p b f", f=H),
                )

            def transpose4(src_sb, tag):
                """4x PE transpose of [128,(4,128)] blocks -> PSUM [128,512]."""
                ps = pp.tile([P, 512], F32, tag=tag)
                for b in range(4):
                    nc.tensor.transpose(
                        ps[:, b * H : (b + 1) * H],
                        src_sb[:, b * H : (b + 1) * H],
                        ident[:],
                    )
                return ps

            def allgather_half(half, buf):
                rows = slice(0, HS) if half == 0 else slice(HS, NS)
                nc.gpsimd.collective_compute(
                    "AllGather",
                    mybir.AluOpType.bypass,
                    replica_groups=groups,
                    ins=[h_sh[rows, :]],
                    outs=[(h_fullA if half == 0 else h_fullB)[buf][:, :]],
                )

            # ---- embedding: h0 = x @ Wemb + bemb (own shard) ----
            for t in range(NT):
                rows = slice(512 * t, 512 * (t + 1))
                xt = load_nm(wp, x_s[rows, :], "in_a")
                ps_x = transpose4(xt, "ps_b")
                xT = wp.tile([P, 512], F32, tag="t_a")
                nc.vector.tensor_copy(xT[:], ps_x[:])
                ps_h = pp.tile([P, 512], F32, tag="ps_mm")
                nc.tensor.matmul(ps_h[:], Wemb_sb[:], xT[:], start=True, stop=True)
                hT = wp.tile([P, 512], F32, tag="t_b")
                nc.vector.tensor_scalar_add(hT[:], ps_h[:], bemb_sb[:, 0:1])
                ps_nm = transpose4(hT, "ps_nm")
                hn = wp.tile([P, 4 * H], F32, tag="out_a")
                nc.vector.tensor_copy(hn[:], ps_nm[:])
                store_nm(hn, h_shF[rows, :])
                hnb = wp.tile([P, 4 * H], BF16, tag="out_b")
                nc.scalar.activation(
                    hnb[:], ps_nm[:], mybir.ActivationFunctionType.Copy
                )
                store_nm(hnb, h_sh[rows, :])
                if t == 12:
                    allgather_half(0, 0)  # rows [0, 6250) done: overlap rest
            allgather_half(1, 0)

            # ---- message-passing layers ----
            def emit_gathers(per_k, buf):
                ets = [None] * NK
                for kk, (s0, nsl) in enumerate(per_k):
                    if nsl == 0:
                        continue
                    gi = sp.tile([P, rowcap * 8], I16, tag=f"gi{kk}")
                    nc.scalar.dma_start(
                        gi[:, : nsl // 16],
                        gidx[:, s0 // 16 : (s0 + nsl) // 16],
                    )
                    et = ep.tile([P, rowcap * H], BF16, tag=f"et{kk}")
                    ets[kk] = et
                    hsrc = (
                        h_fullA[buf][kk * cfg.chunk : (kk + 1) * cfg.chunk, :]
                        if kk < 2
                        else h_fullB[buf][
                            (kk - 2) * cfg.chunk : (kk - 1) * cfg.chunk, :
                        ]
                    )
                    nc.gpsimd.dma_gather(
                        et[:].rearrange("p (r f) -> p r f", f=H)[:, : nsl // 128, :],
                        hsrc,
                        gi[:, : nsl // 16],
                        nsl,
                        nsl,
                        H,
                        single_packet=False,
                        queue_num=kk % nqueues,
                    )
                return ets

            def emit_segsum(ets, blist, wc):
                # psum[f, t] over the window's 4 blocks; batched one-hot
                # generation: one DVE is_equal per SG chunks,
                # S[j, g*128+t] = (iota_t == tcol[j, wc+g])
                ps_seg = pp.tile([P, 512], F32, tag="seg")
                if phase == "noedge":
                    nc.vector.memset(ps_seg[:], 0.0)
                    return ps_seg
                nch_w = sum(len(ch) for ch in blist)
                slabs = []
                for s0 in range(0, nch_w, SG):
                    g = min(SG, nch_w - s0)
                    slab = slb.tile([P, SG * P], BF16, tag="s")
                    nc.scalar.dma_start(
                        slab[:, : g * P],
                        Sslab[:, (wc + s0) * P : (wc + s0 + g) * P],
                    )
                    slabs.append(slab)
                ci = 0
                for j, ch in enumerate(blist):
                    nch = len(ch)
                    for i, (kk, r) in enumerate(ch):
                        S = slabs[ci // SG][:, (ci % SG) * P : (ci % SG + 1) * P]
                        ci += 1
                        ev = ets[kk][:].rearrange("p (r f) -> p r f", f=H)
                        nc.tensor.matmul(
                            ps_seg[:, j * H : (j + 1) * H],
                            ev[:, r, :],
                            S,
                            start=(i == 0),
                            stop=(i == nch - 1),
                        )
                return ps_seg

            def emit_dense(l, w, ps_seg):
                # dense: h = relu(h + G @ W1 + deg*(h @ W2) + deg x bm)
                gT = wp.tile([P, 512], F32, tag="t_a")
                nc.vector.tensor_copy(gT[:], ps_seg[:])
                if phase == "nodense":
                    return
                rows = slice(512 * w, 512 * (w + 1))
                ht = load_nm(wp, h_shF[rows, :], "in_b")
                degT_t = wp.tile([1, 512], F32, tag="degt")
                nc.sync.dma_start(degT_t[:], degT[:, rows])
                dh = wp.tile([P, 4 * H], F32, tag="dh")
                for b in range(4):
                    bs = slice(b * H, (b + 1) * H)
                    nc.vector.tensor_scalar_mul(
                        dh[:, bs], ht[:, bs], deg_sb[:, 4 * w + b : 4 * w + b + 1]
                    )
                ps_d = transpose4(dh, "ps_b")
                dhT = wp.tile([P, 512], F32, tag="t_b")
                nc.scalar.activation(
                    dhT[:], ps_d[:], mybir.ActivationFunctionType.Copy
                )
                ps_mm = pp.tile([P, 512], F32, tag="ps_mm")
                nc.tensor.matmul(ps_mm[:], W1_sb[l][:], gT[:], start=True, stop=False)
                nc.tensor.matmul(ps_mm[:], W2_sb[l][:], dhT[:], start=False, stop=False)
                nc.tensor.matmul(
                    ps_mm[:], bm_sb[l][:], degT_t[:], start=False, stop=True
                )
                mmT = wp.tile([P, 512], F32, tag="t_c")
                nc.scalar.activation(
                    mmT[:], ps_mm[:], mybir.ActivationFunctionType.Copy
                )
                ps_nm = transpose4(mmT, "ps_nm")
                hn = wp.tile([P, 4 * H], F32, tag="out_a")
                nc.vector.tensor_add(hn[:], ht[:], ps_nm[:])
                nc.vector.tensor_scalar_max(hn[:], hn[:], 0.0)
                store_nm(hn, h_shF[rows, :])
                if l < L - 1:
                    hnb = wp.tile([P, 4 * H], BF16, tag="out_b")
                    nc.scalar.activation(
                        hnb[:], hn[:], mybir.ActivationFunctionType.Copy
                    )
                    store_nm(hnb, h_sh[rows, :])

            for l in range(L):
                wc = 0  # window-chunk counter (tcol column, resets per layer)
                prev = None  # (w, ps_seg) one-window pipeline skew
                for w, (per_k, blist) in enumerate(windows):
                    ets = (
                        emit_gathers(per_k, l % 2)
                        if phase != "noedge"
                        else [None] * NK
                    )
                    ps_seg = emit_segsum(ets, blist, wc)
                    wc += sum(len(ch) for ch in blist)
                    if prev is not None:
                        emit_dense(l, prev[0], prev[1])
                        if l < L - 1 and prev[0] == 12:
                            allgather_half(0, (l + 1) % 2)
                    prev = (w, ps_seg)
                emit_dense(l, prev[0], prev[1])
                if l < L - 1:
                    allgather_half(1, (l + 1) % 2)

            # ---- readout: g = per-graph mean (ind already holds 1/count) ----
            ep_cm.__exit__(None, None, None)
            pp_cm.__exit__(None, None, None)
            pp_cm = tc.tile_pool(name="psum_ro", bufs=1, space="PSUM")
            pp = pp_cm.__enter__()
            ro_cm = tc.tile_pool(name="readout", bufs=1)
            ro = ro_cm.__enter__()
            ind_sb = ro.tile([P, NBLK * B], F32, tag="ind")
            nc.sync.dma_start(ind_sb[:], ind[:])
            ps_gr = pp.tile([P, B], F32, tag="ps_gr")
            for t in range(NT):
                rows = slice(512 * t, 512 * (t + 1))
                hro = load_nm(wp, h_shF[rows, :], "in_a")
                for b in range(4):
                    blk = 4 * t + b
                    nc.tensor.matmul(
                        ps_gr[:],
                        hro[:, b * H : (b + 1) * H],
                        ind_sb[:, blk * B : (blk + 1) * B],
                        start=(blk == 0),
                        stop=(blk == NBLK - 1),
                    )
            gp_sb = wp.tile([P, B], F32, tag="gp")
            nc.vector.tensor_copy(gp_sb[:], ps_gr[:])
            nc.sync.dma_start(gpart[:, :], gp_sb[:])
            nc.gpsimd.collective_compute(
                "AllReduce",
                mybir.AluOpType.add,
                replica_groups=groups,
                ins=[gpart[:, :]],
                outs=[gsum[:, :]],
            )
            gs_sb = wp.tile([P, B], F32, tag="gs")
            nc.sync.dma_start(gs_sb[:], gsum[:, :])

            W1r_sb = ro.tile([H, 2 * H], F32, tag="w1r")
            nc.sync.dma_start(W1r_sb[:], W1r[:])
            b1_sb = ro.tile([H, 2], F32, tag="b1r")
            nc.sync.dma_start(b1_sb[:], b1r[:])
            W2ra_sb = ro.tile([H, M], F32, tag="w2ra")
            nc.sync.dma_start(W2ra_sb[:], W2r[0:H, :])
            W2rb_sb = ro.tile([H, M], F32, tag="w2rb")
            nc.sync.dma_start(W2rb_sb[:], W2r[H:, :])
            b2_sb = ro.tile([H, MT], F32, tag="b2r")
            nc.sync.dma_start(b2_sb[:], b2r[:])

            z1 = []
            for i in range(2):
                ps_z = pp.tile([P, B], F32, tag=f"ps_z{i}")
                nc.tensor.matmul(
                    ps_z[:],
                    W1r_sb[:, i * H : (i + 1) * H],
                    gs_sb[:],
                    start=True,
                    stop=True,
                )
                zt = wp.tile([P, B], F32, tag=f"z1_{i}")
                nc.vector.tensor_scalar(
                    zt[:],
                    ps_z[:],
                    b1_sb[:, i : i + 1],
                    0.0,
                    mybir.AluOpType.add,
                    mybir.AluOpType.max,
                )
                z1.append(zt)

            o_sb = wp.tile([P, MT * B], F32, tag="o")
            for m in range(MT):
                ps_o = pp.tile([P, B], F32, tag="ps_o")
                ms = slice(m * H, (m + 1) * H)
                nc.tensor.matmul(
                    ps_o[:], W2ra_sb[:, ms], z1[0][:], start=True, stop=False
                )
                nc.tensor.matmul(
                    ps_o[:], W2rb_sb[:, ms], z1[1][:], start=False, stop=True
                )
                nc.scalar.activation(
                    o_sb[:, m * B : (m + 1) * B],
                    ps_o[:],
                    mybir.ActivationFunctionType.Sigmoid,
                    bias=b2_sb[:, m : m + 1],
                    scale=1.0,
                )
            nc.vector.tensor_scalar_mul(o_sb[:], o_sb[:], TWO_PI)
            nc.sync.dma_start(
                outT[:, :].rearrange("(m p) b -> p m b", p=P),
                o_sb[:].rearrange("p (m b) -> p m b", b=B),
            )
            ro_cm.__exit__(None, None, None)
            pp_cm.__exit__(None, None, None)

    nc.finalize()
    return nc


def _run(inputs, cfg, trace=False):
    x = np.asarray(inputs["x"], np.float32)
    ei = np.asarray(inputs["edge_index"])
    batch = np.asarray(inputs["batch"]).astype(np.int64)
    W_embed = np.asarray(inputs["W_embed"], np.float32)
    b_embed = np.asarray(inputs["b_embed"], np.float32)
    Wm = np.asarray(inputs["Wm"], np.float32)
    bm = np.asarray(inputs["bm"], np.float32)
    W1 = np.asarray(inputs["W1"], np.float32)
    b1 = np.asarray(inputs["b1"], np.float32)
    W2 = np.asarray(inputs["W2"], np.float32)
    b2 = np.asarray(inputs["b2"], np.float32)

    src = np.asarray(ei[0], np.int64)
    tgt = np.asarray(ei[1], np.int64)
    NS, NSP, B, M = cfg.ns, cfg.ns_pad, cfg.B, cfg.M
    NBLK = NSP // 128

    gidx, tcol, windows, rowcap, TOT = _pack_v2(cfg, src, tgt)

    deg = np.bincount(tgt, minlength=cfg.N).astype(np.float32)
    counts = np.bincount(batch, minlength=B).astype(np.float32)
    invc = 1.0 / np.clip(counts, 1.0, None)
    # precomputed one-hot slabs: S[p, c*128+t] = (tcol[p, c] == t), bf16
    tvals = np.arange(P, dtype=np.float32)

    in_maps = []
    for c in range(NCORES):
        sl = slice(c * NS, (c + 1) * NS)
        x_c = np.zeros((NSP, H), np.float32)
        x_c[:NS] = x[sl]
        deg_c = np.zeros(NSP, np.float32)
        deg_c[:NS] = deg[sl]
        ind_c = np.zeros((NSP, B), np.float32)
        ind_c[np.arange(NS), batch[sl]] = invc[batch[sl]]
        in_maps.append(
            {
                "x_s": x_c,
                "Wemb": W_embed,
                "bemb": b_embed.reshape(H, 1),
                "Wm": Wm,
                "bmv": bm.reshape(cfg.L, 1, H),
                "W1r": W1,
                "b1r": np.ascontiguousarray(b1.reshape(2, H).T),
                "W2r": W2,
                "b2r": np.ascontiguousarray(b2.reshape(M // 128, H).T),
                "gidx": gidx[c],
                "Sslab": np.ascontiguousarray(
                    (tcol[c][:, :, None] == tvals[None, None, :])
                    .astype(ml_dtypes.bfloat16)
                    .reshape(P, -1)
                ),
                "deg_pb": np.ascontiguousarray(deg_c.reshape(NBLK, P).T),
                "degT": deg_c.reshape(1, NSP),
                "ind": np.ascontiguousarray(
                    ind_c.reshape(NBLK, P, B).transpose(1, 0, 2).reshape(P, NBLK * B)
                ),
            }
        )

    nc = _build(cfg, TOT, windows, rowcap)
    res = run_bass_kernel_spmd(
        nc, in_maps, core_ids=list(range(NCORES)), trace=trace
    )
    out = np.ascontiguousarray(res.results[0]["outT"].T)
    return out, res


def kernel(**inputs) -> np.ndarray:
    cfg = Cfg(N=100000, E=1600000, B=16, M=2048, L=3, chunk=25000, blk=6144)
    trace = bool(os.environ.get("GNN_TRACE"))
    out, _ = _run(inputs, cfg, trace=trace)
    return out
